# revision 7
# baseline (speedup 1.0000x reference)
"""Trainium2 Bass kernel for nn_Decoder (pointer-generator decoder step).

Data-parallel over B=2048 across 8 NeuronCores (256 rows/core, 2 blocks of
128 rows each). Everything runs on-device except cheap index/layout prep:
- host pre-transposes weights and enc_out layouts (pure data movement),
- host resolves duplicate scatter indices (keep-last) into a 0/1 mask.

Per 128-row block the device computes: context (Pool mult + DVE reduce),
embedding row gather (indirect DMA), one LSTM step (PE + ACT + DVE),
attention energies (PE matmuls with the coverage and bias terms folded in
as extra contraction rows), softmax over S, coverage outputs, the V=50257
softmax (PE matmul -> ACT exp with accumulated row sums -> DVE scale),
dense output write (SWDGE cast DMA bf16->f32) and the copy-attention
scatter (per-s indirect scatter-add DMAs into the dense output).
"""

import numpy as np
import ml_dtypes

import bass_rust
import concourse.bass as bass
import concourse.tile as tile
from concourse import mybir
from concourse.bass_utils import run_bass_kernel_spmd
from concourse.masks import make_identity

BF16 = mybir.dt.bfloat16
F32 = mybir.dt.float32
I32 = mybir.dt.int32

B, S, V = 2048, 200, 50257
E, H, EMB = 60, 30, 20
NCORES = 8
RPC = B // NCORES          # rows per core (256)
P = 128                    # rows per block
NBLK = RPC // P            # 2
SCHUNK = 25                # s per lhsT load chunk
NSCH = S // SCHUNK         # 8
SG = 17                    # s per attention psum group
NSG = (S + SG - 1) // SG   # 12 groups (11x17 + 13)
VC = 1024                  # vocab psum chunk
NVC = (V + VC - 1) // VC   # 50 chunks (49x1024 + 81)
VHALF = 25 * VC            # 25600 cols in expbuf half 0
VH2 = V - VHALF            # 24657 in half 1
VWCH = 4096                # v_wT load chunk width
XW = E + EMB               # 80
PGW = E + EMB + H          # 110


def _split_multiwaits(nc):
    """This walrus build rejects >1 sync-wait per instruction; hoist extras
    onto same-engine NoOps placed immediately before the instruction."""
    k = 0
    for bb in nc.m.functions[0].blocks:
        insts = list(bb.instructions)
        out = []
        changed = False
        for inst in insts:
            si = inst.sync_info
            waits = list(si.on_wait) if si and si.on_wait else []
            if len(waits) > 1:
                ups = list(si.on_update) if si and si.on_update else []
                for w in waits[:-1]:
                    k += 1
                    n = mybir.InstNoOp(name=f"I-mwsplit-{k}", ins=[], outs=[])
                    n.engine = inst.engine
                    n.sync_info = bass_rust.SyncInfo(on_wait=[w], on_update=[])
                    out.append(n)
                inst.sync_info = bass_rust.SyncInfo(on_wait=[waits[-1]], on_update=ups)
                changed = True
            out.append(inst)
        if changed:
            bb.instructions = out


def _bcast(ap, mid, inner):
    """[P, inner] AP -> [P, mid, inner] with the middle dim broadcast."""
    return bass.AP(tensor=ap.tensor, offset=ap.offset,
                   ap=[ap.ap[0], [0, mid], ap.ap[1]])


def _build_program():
    nc = bass.Bass(trn_type="TRN2", num_devices=NCORES)

    # ---- I/O ----
    enc_ctx = nc.dram_tensor("enc_ctx", [NBLK, P, E, S], BF16, kind="ExternalInput")
    enc_lhsT = nc.dram_tensor("enc_lhsT", [NBLK, NSCH, E + 2, SCHUNK, P], F32,
                              kind="ExternalInput")
    cov_in = nc.dram_tensor("cov", [NBLK, P, S], F32, kind="ExternalInput")
    attn_in = nc.dram_tensor("attn", [NBLK, P, S], F32, kind="ExternalInput")
    h0_in = nc.dram_tensor("h0", [NBLK, P, H], F32, kind="ExternalInput")
    c0_in = nc.dram_tensor("c0", [NBLK, P, H], F32, kind="ExternalInput")
    dec_in = nc.dram_tensor("dec", [NBLK, P, 1], I32, kind="ExternalInput")
    emb_in = nc.dram_tensor("emb", [V, EMB], F32, kind="ExternalInput")
    vwt_in = nc.dram_tensor("vwT", [H + 1, V], BF16, kind="ExternalInput")
    rhsa_in = nc.dram_tensor("rhs_aug", [E + 2, H], F32, kind="ExternalInput")
    wst_in = nc.dram_tensor("ws_tiled", [H, SG * H], F32, kind="ExternalInput")
    wih_in = nc.dram_tensor("wihT", [XW + 1, 4 * H], F32, kind="ExternalInput")
    whh_in = nc.dram_tensor("whhT", [H, 4 * H], F32, kind="ExternalInput")
    pw_in = nc.dram_tensor("pw", [1, PGW], F32, kind="ExternalInput")
    vvec_in = nc.dram_tensor("vvec", [1, H], F32, kind="ExternalInput")
    sidx_in = nc.dram_tensor("sidx", [NBLK, P, S], I32, kind="ExternalInput")
    smask_in = nc.dram_tensor("smask", [NBLK, P, S], F32, kind="ExternalInput")

    outs = [nc.dram_tensor(f"out{b}", [P * V, 1], F32, kind="ExternalOutput")
            for b in range(NBLK)]
    nattn_out = nc.dram_tensor("nattn", [NBLK, P, S], F32, kind="ExternalOutput")
    ncov_out = nc.dram_tensor("ncov", [NBLK, P, S], F32, kind="ExternalOutput")
    h_out = nc.dram_tensor("hout", [NBLK, P, H], F32, kind="ExternalOutput")
    c_out = nc.dram_tensor("cout", [NBLK, P, H], F32, kind="ExternalOutput")
    cl_out = nc.dram_tensor("clp", [NBLK, P, 1], F32, kind="ExternalOutput")

    with tile.TileContext(nc) as tc:
        with (
            tc.tile_pool(name="consts", bufs=1) as consts,
            tc.tile_pool(name="expb", bufs=1) as expb,
            tc.tile_pool(name="ctxp", bufs=1) as ctxp,
            tc.tile_pool(name="lhsp", bufs=2) as lhsp,
            tc.tile_pool(name="vwp", bufs=2) as vwp,
            tc.tile_pool(name="smp", bufs=2) as smp,
            tc.tile_pool(name="pm", bufs=2, space="PSUM") as pmp,
            tc.tile_pool(name="pv", bufs=2, space="PSUM") as pvp,
            tc.tile_pool(name="psm", bufs=1, space="PSUM") as psm,
        ):
            # ---- constants ----
            ident = consts.tile([P, P], F32)
            make_identity(nc, ident[:])
            rhsa = consts.tile([E + 2, H], F32)
            nc.sync.dma_start(out=rhsa[:], in_=rhsa_in[:])
            wst = consts.tile([H, SG * H], F32)
            nc.sync.dma_start(out=wst[:], in_=wst_in[:])
            wih = consts.tile([XW + 1, 4 * H], F32)
            nc.sync.dma_start(out=wih[:], in_=wih_in[:])
            whh = consts.tile([H, 4 * H], F32)
            nc.sync.dma_start(out=whh[:], in_=whh_in[:])
            pw_rep = consts.tile([P, PGW], F32)
            nc.sync.dma_start(
                out=pw_rep[:],
                in_=bass.AP(tensor=pw_in.ap().tensor, offset=0,
                            ap=[[0, P], [1, PGW]]))
            v_rep = consts.tile([P, H], F32)
            nc.sync.dma_start(
                out=v_rep[:],
                in_=bass.AP(tensor=vvec_in.ap().tensor, offset=0,
                            ap=[[0, P], [1, H]]))

            # persistent vocab-exp buffers (shared across blocks)
            ebuf0 = expb.tile([P, VHALF], BF16, tag="e0", name="ebuf0")
            ebuf1 = expb.tile([P, VH2], BF16, tag="e1", name="ebuf1")
            ebufs = [ebuf0, ebuf1]

            for blk in range(NBLK):
                # ---------- loads ----------
                ctx_t = ctxp.tile([P, E * S], BF16, tag="ctx")
                nc.sync.dma_start(out=ctx_t[:],
                                  in_=enc_ctx[blk].rearrange("p e s -> p (e s)"))
                cov_t = smp.tile([P, S], F32, tag="cov")
                nc.sync.dma_start(out=cov_t[:], in_=cov_in[blk])
                attn_t = smp.tile([P, S], F32, tag="attn")
                nc.sync.dma_start(out=attn_t[:], in_=attn_in[blk])
                h0_t = smp.tile([P, H], F32, tag="h0")
                nc.sync.dma_start(out=h0_t[:], in_=h0_in[blk])
                c0_t = smp.tile([P, H], F32, tag="c0")
                nc.sync.dma_start(out=c0_t[:], in_=c0_in[blk])
                dec_t = smp.tile([P, 1], I32, tag="dec")
                nc.sync.dma_start(out=dec_t[:], in_=dec_in[blk])
                sidx_t = smp.tile([P, S], I32, tag="sidx")
                nc.sync.dma_start(out=sidx_t[:], in_=sidx_in[blk])
                smask_t = smp.tile([P, S], F32, tag="smask")
                nc.sync.dma_start(out=smask_t[:], in_=smask_in[blk])

                # ---------- context ----------
                attn_b = bass.AP(tensor=attn_t[:].tensor, offset=attn_t[:].offset,
                                 ap=[attn_t[:].ap[0], [0, E], [1, S]])
                nc.gpsimd.tensor_tensor(
                    out=ctx_t[:], in0=ctx_t[:],
                    in1=attn_b, op=mybir.AluOpType.mult)
                ctx_sb = smp.tile([P, E], F32, tag="ctxsb")
                nc.vector.reduce_sum(
                    ctx_sb[:], ctx_t[:].rearrange("p (e s) -> p e s", s=S),
                    axis=mybir.AxisListType.X)

                # ---------- embedding gather ----------
                emb_t = smp.tile([P, EMB], F32, tag="embt")
                nc.gpsimd.indirect_dma_start(
                    out=emb_t[:], out_offset=None,
                    in_=emb_in[:],
                    in_offset=bass.IndirectOffsetOnAxis(ap=dec_t[:, :1], axis=0))

                # ---------- x = [ctx, embed, ones]; transposes ----------
                x_t = smp.tile([P, XW + 1], F32, tag="xt")
                nc.vector.tensor_copy(x_t[:, 0:E], ctx_sb[:])
                nc.vector.tensor_copy(x_t[:, E:XW], emb_t[:])
                nc.vector.memset(x_t[:, XW:XW + 1], 1.0)
                xT_ps = psm.tile([XW + 1, P], F32, tag="ps_t", space="PSUM")
                nc.tensor.transpose(out=xT_ps[:], in_=x_t[:], identity=ident[:])
                xT = smp.tile([XW + 1, P], F32, tag="xT")
                nc.vector.tensor_copy(xT[:], xT_ps[:])
                h0T_ps = psm.tile([H, P], F32, tag="ps_t", space="PSUM")
                nc.tensor.transpose(out=h0T_ps[:], in_=h0_t[:], identity=ident[:])
                h0T = smp.tile([H, P], F32, tag="h0T")
                nc.vector.tensor_copy(h0T[:], h0T_ps[:])

                # ---------- LSTM gates ----------
                gates = psm.tile([P, 4 * H], F32, tag="ps_g", space="PSUM")
                nc.tensor.matmul(gates[:], lhsT=xT[:], rhs=wih[:],
                                 start=True, stop=False)
                nc.tensor.matmul(gates[:], lhsT=h0T[:], rhs=whh[:],
                                 start=False, stop=True)

                def sigmoid_from(psl, tag):
                    t = smp.tile([P, H], F32, tag=tag)
                    nc.scalar.activation(t[:], psl,
                                         mybir.ActivationFunctionType.Exp,
                                         scale=-1.0)
                    nc.vector.tensor_scalar_add(t[:], t[:], 1.0)
                    nc.vector.reciprocal(t[:], t[:])
                    return t

                sig_i = sigmoid_from(gates[:, 0:H], "sigi")
                sig_f = sigmoid_from(gates[:, H:2 * H], "sigf")
                tg = smp.tile([P, H], F32, tag="tg")
                nc.scalar.activation(tg[:], gates[:, 2 * H:3 * H],
                                     mybir.ActivationFunctionType.Tanh)
                sig_o = sigmoid_from(gates[:, 3 * H:4 * H], "sigo")

                c_new = smp.tile([P, H], F32, tag="cnew")
                nc.vector.tensor_mul(c_new[:], sig_f[:], c0_t[:])
                igt = smp.tile([P, H], F32, tag="igt")
                nc.vector.tensor_mul(igt[:], sig_i[:], tg[:])
                nc.vector.tensor_add(c_new[:], c_new[:], igt[:])
                tc_t = smp.tile([P, H], F32, tag="tct")
                nc.scalar.activation(tc_t[:], c_new[:],
                                     mybir.ActivationFunctionType.Tanh)
                h_new = smp.tile([P, H], F32, tag="hnew")
                nc.vector.tensor_mul(h_new[:], sig_o[:], tc_t[:])
                nc.sync.dma_start(out=h_out[blk], in_=h_new[:])
                nc.sync.dma_start(out=c_out[blk], in_=c_new[:])

                # transposed h (f32 for attention fold, bf16+ones for vocab)
                haug_in = smp.tile([P, H + 1], F32, tag="haugin")
                nc.vector.tensor_copy(haug_in[:, 0:H], h_new[:])
                nc.vector.memset(haug_in[:, H:H + 1], 1.0)
                hT_ps = psm.tile([H + 1, P], F32, tag="ps_t", space="PSUM")
                nc.tensor.transpose(out=hT_ps[:], in_=haug_in[:], identity=ident[:])
                hT = smp.tile([H, P], F32, tag="hT")
                nc.vector.tensor_copy(hT[:], hT_ps[0:H, :])
                haug = smp.tile([H + 1, P], BF16, tag="haug")
                nc.vector.tensor_copy(haug[:], hT_ps[:])

                # ---------- p_gen ----------
                xcat = smp.tile([P, PGW], F32, tag="xcat")
                nc.vector.tensor_copy(xcat[:, 0:XW], x_t[:, 0:XW])
                nc.vector.tensor_copy(xcat[:, XW:PGW], h_new[:])
                junk = smp.tile([P, PGW], F32, tag="junk")
                pgd = smp.tile([P, 1], F32, tag="pgd")
                nc.vector.scalar_tensor_tensor(
                    out=junk[:], in0=xcat[:], scalar=1.0, in1=pw_rep[:],
                    op0=mybir.AluOpType.mult, op1=mybir.AluOpType.mult,
                    accum_out=pgd[:])
                pg = smp.tile([P, 1], F32, tag="pg")
                nc.scalar.activation(pg[:], pgd[:],
                                     mybir.ActivationFunctionType.Exp, scale=-1.0)
                nc.vector.tensor_scalar_add(pg[:], pg[:], 1.0)
                nc.vector.reciprocal(pg[:], pg[:])
                om = smp.tile([P, 1], F32, tag="om")  # 1 - p_gen
                nc.vector.tensor_scalar(
                    out=om[:], in0=pg[:], scalar1=-1.0, scalar2=1.0,
                    op0=mybir.AluOpType.mult, op1=mybir.AluOpType.add)

                # ---------- attention energies ----------
                # ws_app = h @ attn_ws_w.T  (added per s-group via broadcast)
                wsap_ps = psm.tile([P, H], F32, tag="ps_g", space="PSUM")
                nc.tensor.matmul(wsap_ps[:], lhsT=hT[:], rhs=wst[:, 0:H],
                                 start=True, stop=True)
                wsap = smp.tile([P, H], F32, tag="wsap")
                nc.vector.tensor_copy(wsap[:], wsap_ps[:])

                lhsT_tiles = []
                for ch in range(NSCH):
                    lt = lhsp.tile([E + 2, SCHUNK * P], F32, tag="lhsT")
                    nc.sync.dma_start(
                        out=lt[:],
                        in_=enc_lhsT[blk, ch].rearrange("k s p -> k (s p)"))
                    lhsT_tiles.append(lt)
                energy = smp.tile([P, S], F32, tag="energy")
                for g in range(NSG):
                    s0 = g * SG
                    ns = min(SG, S - s0)
                    pm = pmp.tile([P, SG * H], F32, tag="pm", space="PSUM")
                    for si in range(ns):
                        s = s0 + si
                        lt = lhsT_tiles[s // SCHUNK]
                        sl = s % SCHUNK
                        lhs_ap = lt[:].rearrange("k (s p) -> k s p", p=P)[:, sl, :]
                        nc.tensor.matmul(pm[:, si * H:(si + 1) * H],
                                         lhsT=lhs_ap, rhs=rhsa[:],
                                         start=True, stop=True)
                    t_t = smp.tile([P, SG * H], F32, tag="tanh")
                    nc.vector.scalar_tensor_tensor(
                        out=t_t[:, 0:ns * H], in0=pm[:, 0:ns * H], scalar=1.0,
                        in1=_bcast(wsap[:], ns, H),
                        op0=mybir.AluOpType.mult, op1=mybir.AluOpType.add)
                    nc.scalar.activation(t_t[:, 0:ns * H], t_t[:, 0:ns * H],
                                         mybir.ActivationFunctionType.Tanh)
                    nc.vector.scalar_tensor_tensor(
                        out=t_t[:, 0:ns * H], in0=t_t[:, 0:ns * H], scalar=1.0,
                        in1=_bcast(v_rep[:], ns, H),
                        op0=mybir.AluOpType.mult, op1=mybir.AluOpType.mult)
                    nc.vector.reduce_sum(
                        energy[:, s0:s0 + ns],
                        t_t[:, 0:ns * H].rearrange("p (s h) -> p s h", h=H),
                        axis=mybir.AxisListType.X)

                # ---------- softmax over S; coverage outputs ----------
                emax = smp.tile([P, 1], F32, tag="emax")
                nc.vector.reduce_max(emax[:], energy[:], axis=mybir.AxisListType.X)
                negmax = smp.tile([P, 1], F32, tag="negmax")
                nc.vector.tensor_scalar_mul(negmax[:], emax[:], -1.0)
                nattn_t = smp.tile([P, S], F32, tag="nattn")
                esum = smp.tile([P, 1], F32, tag="esum")
                nc.scalar.activation(nattn_t[:], energy[:],
                                     mybir.ActivationFunctionType.Exp,
                                     bias=negmax[:, :1], accum_out=esum[:])
                esr = smp.tile([P, 1], F32, tag="esr")
                nc.vector.reciprocal(esr[:], esum[:])
                nc.vector.tensor_scalar_mul(nattn_t[:], nattn_t[:], esr[:, :1])
                nc.sync.dma_start(out=nattn_out[blk], in_=nattn_t[:])
                ncov_t = smp.tile([P, S], F32, tag="ncov")
                nc.vector.tensor_add(ncov_t[:], cov_t[:], nattn_t[:])
                nc.sync.dma_start(out=ncov_out[blk], in_=ncov_t[:])
                clmin = smp.tile([P, S], F32, tag="clmin")
                nc.vector.tensor_tensor(out=clmin[:], in0=nattn_t[:],
                                        in1=cov_t[:], op=mybir.AluOpType.min)
                clp = smp.tile([P, 1], F32, tag="clp")
                nc.vector.reduce_sum(clp[:], clmin[:], axis=mybir.AxisListType.X)
                nc.sync.dma_start(out=cl_out[blk], in_=clp[:])

                # scatter values = (1 - p_gen) * new_attn * keepmask
                svals = smp.tile([P, S], F32, tag="svals")
                nc.vector.scalar_tensor_tensor(
                    out=svals[:], in0=nattn_t[:], scalar=om[:, :1], in1=smask_t[:],
                    op0=mybir.AluOpType.mult, op1=mybir.AluOpType.mult)

                # ---------- vocab softmax ----------
                parts = smp.tile([P, NVC], F32, tag="parts")
                vw_tiles = []
                for ch in range((V + VWCH - 1) // VWCH):
                    w = min(VWCH, V - ch * VWCH)
                    vt = vwp.tile([H + 1, VWCH], BF16, tag="vw")
                    nc.sync.dma_start(out=vt[:, 0:w],
                                      in_=vwt_in[:, ch * VWCH:ch * VWCH + w])
                    vw_tiles.append(vt)
                for ch in range(NVC):
                    c0 = ch * VC
                    w = min(VC, V - c0)
                    pv = pvp.tile([P, VC], F32, tag="pv", space="PSUM")
                    for half in range(2):
                        n0 = c0 + half * 512
                        n = min(512, V - n0, VC - half * 512)
                        if n <= 0:
                            continue
                        vt = vw_tiles[n0 // VWCH]
                        off = n0 % VWCH
                        nc.tensor.matmul(pv[:, half * 512:half * 512 + n],
                                         lhsT=haug[:], rhs=vt[:, off:off + n],
                                         start=True, stop=True)
                    if c0 < VHALF:
                        eslice = ebufs[0][:, c0:c0 + w]
                    else:
                        eslice = ebufs[1][:, c0 - VHALF:c0 - VHALF + w]
                    nc.scalar.activation(eslice, pv[:, 0:w],
                                         mybir.ActivationFunctionType.Exp,
                                         accum_out=parts[:, ch:ch + 1])
                stot = smp.tile([P, 1], F32, tag="stot")
                nc.vector.reduce_sum(stot[:], parts[:], axis=mybir.AxisListType.X)
                scal = smp.tile([P, 1], F32, tag="scal")
                nc.vector.reciprocal(scal[:], stot[:])
                nc.vector.tensor_mul(scal[:], scal[:], pg[:])

                out_d = outs[blk]
                out2d = out_d[:].rearrange("(p v) one -> p (v one)", p=P)
                for hf, w0 in ((0, 0), (1, VHALF)):
                    eb = ebufs[hf]
                    wdt = VHALF if hf == 0 else VH2
                    nc.vector.tensor_scalar_mul(eb[:], eb[:], scal[:, :1])
                    nc.gpsimd.dma_start(out=out2d[:, w0:w0 + wdt], in_=eb[:])

                # ---------- copy-attention scatter ----------
                for s in range(S):
                    nc.gpsimd.indirect_dma_start(
                        out=out_d[:],
                        out_offset=bass.IndirectOffsetOnAxis(
                            ap=sidx_t[:, s:s + 1], axis=0),
                        in_=svals[:, s:s + 1],
                        in_offset=None,
                        compute_op=mybir.AluOpType.add)

    _split_multiwaits(nc)
    return nc


_NC = None


def _get_program():
    global _NC
    if _NC is None:
        _NC = _build_program()
    return _NC


def _prep_core_inputs(core, coverage, enc_out, h0, c0, attn, dec_input,
                      enc_inputs, embedding, shared):
    r0 = core * RPC
    rows = slice(r0, r0 + RPC)
    bf16 = ml_dtypes.bfloat16

    eo = np.ascontiguousarray(enc_out[rows])            # [256, S, E] f32
    eo_blk = eo.reshape(NBLK, P, S, E)
    # ctx layout [blk, p, e, s]
    enc_ctx = np.ascontiguousarray(
        eo_blk.transpose(0, 1, 3, 2)).astype(bf16)
    # lhsT layout [blk, chunk, e+2, s_in, p]
    t = eo_blk.reshape(NBLK, P, NSCH, SCHUNK, E).transpose(0, 2, 4, 3, 1)
    cov_blk = coverage[rows].reshape(NBLK, P, NSCH, SCHUNK)
    covT = cov_blk.transpose(0, 2, 3, 1)[:, :, None, :, :]   # [blk, ch, 1, s, p]
    ones = np.ones((NBLK, NSCH, 1, SCHUNK, P), np.float32)
    enc_lhsT = np.ascontiguousarray(
        np.concatenate([t, covT, ones], axis=2), dtype=np.float32)

    dec = dec_input[rows].astype(np.int32).reshape(NBLK, P, 1)

    enc_i = enc_inputs[rows].astype(np.int64)            # [256, S]
    sidx = np.empty((NBLK, P, S), np.int32)
    smask = np.ones((NBLK, P, S), np.float32)
    for blk in range(NBLK):
        cols = enc_i[blk * P:(blk + 1) * P]              # [P, S]
        flat = (np.arange(P, dtype=np.int64)[:, None] * V + cols)
        # keep-last duplicate resolution: drop s if a later s' hits same col
        for p in range(P):
            seen = {}
            cp = cols[p]
            for s in range(S - 1, -1, -1):
                if cp[s] in seen:
                    smask[blk, p, s] = 0.0
                    flat[p, s] = p * V     # harmless: adds 0.0 there
                else:
                    seen[cp[s]] = s
        sidx[blk] = flat.astype(np.int32)

    return {
        "enc_ctx": enc_ctx,
        "enc_lhsT": enc_lhsT,
        "cov": coverage[rows].reshape(NBLK, P, S).astype(np.float32),
        "attn": attn[rows].reshape(NBLK, P, S).astype(np.float32),
        "h0": h0[0, rows].reshape(NBLK, P, H).astype(np.float32),
        "c0": c0[0, rows].reshape(NBLK, P, H).astype(np.float32),
        "dec": dec,
        "emb": shared["emb"],
        "vwT": shared["vwT"],
        "rhs_aug": shared["rhs_aug"],
        "ws_tiled": shared["ws_tiled"],
        "wihT": shared["wihT"],
        "whhT": shared["whhT"],
        "pw": shared["pw"],
        "vvec": shared["vvec"],
        "sidx": sidx,
        "smask": smask,
    }


def kernel(coverage, enc_out, h0, c0, attn, dec_input, enc_inputs, embedding,
           w_ih, w_hh, b_ih, b_hh, attn_wh_w, attn_wh_b, attn_ws_w, attn_ws_b,
           attn_wc, attn_v, wh, ws, wx, v_w, v_b):
    bf16 = ml_dtypes.bfloat16
    coverage = np.asarray(coverage, np.float32)
    enc_out = np.asarray(enc_out, np.float32)
    h0 = np.asarray(h0, np.float32)
    c0 = np.asarray(c0, np.float32)
    attn = np.asarray(attn, np.float32)
    dec_input = np.asarray(dec_input)
    enc_inputs = np.asarray(enc_inputs)
    embedding = np.asarray(embedding, np.float32)

    # shared (replicated) weight prep
    rhs_aug = np.concatenate([
        np.asarray(attn_wh_w, np.float32).T,            # [E, H]
        np.asarray(attn_wc, np.float32)[None, :],       # wc row (x coverage)
        (np.asarray(attn_wh_b, np.float32)
         + np.asarray(attn_ws_b, np.float32))[None, :],  # bias row (x ones)
    ], axis=0)                                           # [E+2, H]
    ws_tiled = np.tile(np.asarray(attn_ws_w, np.float32).T, (1, SG))  # [H, SG*H]
    wihT = np.concatenate([
        np.asarray(w_ih, np.float32).T,                  # [XW, 4H]
        (np.asarray(b_ih, np.float32)
         + np.asarray(b_hh, np.float32))[None, :],       # bias row
    ], axis=0)                                           # [XW+1, 4H]
    whhT = np.ascontiguousarray(np.asarray(w_hh, np.float32).T)  # [H, 4H]
    pw = np.concatenate([np.asarray(wh, np.float32),
                         np.asarray(wx, np.float32),
                         np.asarray(ws, np.float32)])[None, :]   # [1, 110]
    vwT = np.ascontiguousarray(np.concatenate(
        [np.asarray(v_w, np.float32).T,
         np.asarray(v_b, np.float32)[None, :]], axis=0)).astype(bf16)  # [31, V]
    shared = {
        "emb": embedding,
        "vwT": vwT,
        "rhs_aug": np.ascontiguousarray(rhs_aug),
        "ws_tiled": np.ascontiguousarray(ws_tiled),
        "wihT": np.ascontiguousarray(wihT),
        "whhT": whhT,
        "pw": np.ascontiguousarray(pw),
        "vvec": np.ascontiguousarray(np.asarray(attn_v, np.float32)[None, :]),
    }

    in_maps = [
        _prep_core_inputs(cc, coverage, enc_out, h0, c0, attn, dec_input,
                          enc_inputs, embedding, shared)
        for cc in range(NCORES)
    ]

    nc = _get_program()
    res = run_bass_kernel_spmd(nc, in_maps, list(range(NCORES)))

    # ---- assemble full outputs ----
    out_blocks = []
    nattn_l, ncov_l, h_l, c_l = [], [], [], []
    cl_total = np.float64(0.0)
    for cc in range(NCORES):
        r = res.results[cc]
        for blk in range(NBLK):
            out_blocks.append(r[f"out{blk}"].reshape(P, V))
        nattn_l.append(r["nattn"].reshape(RPC, S))
        ncov_l.append(r["ncov"].reshape(RPC, S))
        h_l.append(r["hout"].reshape(RPC, H))
        c_l.append(r["cout"].reshape(RPC, H))
        cl_total += np.float64(r["clp"].sum())

    output = np.concatenate(out_blocks, axis=0)
    new_attn = np.concatenate(nattn_l, axis=0)
    new_cov = np.concatenate(ncov_l, axis=0)
    h_full = np.concatenate(h_l, axis=0)[None]
    c_full = np.concatenate(c_l, axis=0)[None]
    cov_loss = np.float32(cl_total)
    return output, new_cov, (h_full, c_full), new_attn, cov_loss
